# revision 26
# baseline (speedup 1.0000x reference)
"""BipartiteGCN message-passing kernel for 8 TRN2 NeuronCores.

Math:  out = D_c^{-1/2} A^T D_r^{-1/2} (x @ W) + b
where A[s, d] = multiplicity of edge (gene s, drug d), s, d in [0, 4000).

Strategy (dst-window sharding, no output all-reduce):
  - Core c owns drug (dst) window [512c, 512c+512).  Edges are sharded to
    cores by dst window and sorted by src gene (host-side layout only; all
    arithmetic happens on device).
  - Each core builds its dense count stripe A_c [4096 genes x 512 drugs]
    directly in SBUF with one-hot x one-hot PE matmuls: for each 128-edge
    chunk, lhsT[e, g] = (src_e == g), rhs[e, d] = (dst_e == d) (fp16
    one-hots built by DVE compare-vs-iota), accumulated per 128-gene window
    in fp32 PSUM.  No gather/scatter DMA at all.
  - xW is computed row-sharded over genes (512 rows/core) and all-gathered.
  - row_deg = free-axis rowsums of A_c (partial -> 16KB AllReduce);
    col_deg = ones^T @ A_c on the PE (local).  f = rsqrt-masked row_deg,
    g = rsqrt-masked col_deg, all on device.
  - out_c = g * ((f*A_c)^T @ xWf) + bias ; host concatenates the stripes.
"""

import sys

if "/opt/trn_rl_repo" not in sys.path:
    sys.path.insert(0, "/opt/trn_rl_repo")

import numpy as np

import concourse.bass as bass  # noqa: F401
import concourse.mybir as mybir
from concourse import bacc, tile

CORES = 8
DWIN = 512              # dst (drug) window per core
ND = 4000               # number of drugs
GD = 4096               # padded gene dim (src < 4000)
IC = 1024
OC = 512
ST = GD // 128          # 32 gene windows / tiles
WCH = 10                # 128-edge chunks per gene window (max 1172 edges)
NCH = ST * WCH          # 320 chunks per core
NSLOT = NCH * 128       # 40960 edge slots per core

F32 = mybir.dt.float32
F16 = mybir.dt.float16


def build_nc(debug_outputs=False):
    nc = bacc.Bacc(
        None,
        target_bir_lowering=False,
        debug=False,
        num_devices=CORES,
    )

    xT = nc.dram_tensor("xT", [IC, DWIN], F32, kind="ExternalInput")
    w = nc.dram_tensor("w", [IC, OC], F32, kind="ExternalInput")
    brep = nc.dram_tensor("brep", [128, OC], F32, kind="ExternalInput")
    i128 = nc.dram_tensor("i128", [128, 128], F16, kind="ExternalInput")
    i512 = nc.dram_tensor("i512", [128, OC], F16, kind="ExternalInput")
    sloc = nc.dram_tensor("sloc", [128, NCH], F32, kind="ExternalInput")
    dloc = nc.dram_tensor("dloc", [128, NCH], F32, kind="ExternalInput")
    out = nc.dram_tensor("out", [DWIN, OC], F32, kind="ExternalOutput")

    xw0l = nc.dram_tensor("xw0l", [DWIN, OC], F32)         # local xW stripe
    xw0f = nc.dram_tensor("xw0f", [GD, OC], F32, addr_space="Shared")
    rdl = nc.dram_tensor("rdl", [128, ST], F32)            # rowdeg partial
    rds = nc.dram_tensor("rds", [128, ST], F32, addr_space="Shared")

    Adbg = xwdbg = None
    if debug_outputs:
        Adbg = nc.dram_tensor("Adbg", [GD, OC], F32, kind="ExternalOutput")
        xwdbg = nc.dram_tensor("xwdbg", [GD, OC], F32, kind="ExternalOutput")

    with tile.TileContext(nc) as tc:
        with (
            tc.tile_pool(name="const", bufs=1) as cpool,
            tc.tile_pool(name="work", bufs=2) as wpool,
            tc.tile_pool(name="apool", bufs=ST) as apool,
            tc.tile_pool(name="psum", bufs=4, space="PSUM") as ppool,
        ):
            # constants
            ones_sb = cpool.tile([128, 1], F32)
            nc.vector.memset(ones_sb[:], 1.0)
            i128_sb = cpool.tile([128, 128], F16)
            nc.sync.dma_start(i128_sb[:], i128[:])
            i512_sb = cpool.tile([128, OC], F16)
            nc.sync.dma_start(i512_sb[:], i512[:])
            bias_sb = cpool.tile([128, OC], F32)
            nc.sync.dma_start(bias_sb[:], brep[:])
            sloc_sb = cpool.tile([128, NCH], F32)
            nc.sync.dma_start(sloc_sb[:], sloc[:])
            dloc_sb = cpool.tile([128, NCH], F32)
            nc.sync.dma_start(dloc_sb[:], dloc[:])

            # phase B: xw0 = x_shard @ W  (genes 512c..512c+512)
            pb = [ppool.tile([128, OC], F32, tag="acc", name=f"pb{i}") for i in range(4)]
            for kt in range(8):
                xt_t = wpool.tile([128, DWIN], F32, tag="xT", name=f"xt{kt}")
                w_t = wpool.tile([128, OC], F32, tag="w", name=f"w{kt}")
                nc.sync.dma_start(xt_t[:], xT[kt * 128:(kt + 1) * 128, :])
                nc.sync.dma_start(w_t[:], w[kt * 128:(kt + 1) * 128, :])
                for mt in range(4):
                    nc.tensor.matmul(
                        pb[mt][:],
                        xt_t[:, mt * 128:(mt + 1) * 128],
                        w_t[:],
                        start=(kt == 0),
                        stop=(kt == 7),
                    )
            for mt in range(4):
                o = wpool.tile([128, OC], F32, tag="xw0sb", name=f"xw0sb{mt}")
                nc.vector.tensor_copy(o[:], pb[mt][:])
                nc.sync.dma_start(xw0l[mt * 128:(mt + 1) * 128, :], o[:])

            # phase C: all-gather xW  (rank r -> rows 512r..512r+512)
            nc.gpsimd.collective_compute(
                "AllGather",
                mybir.AluOpType.bypass,
                replica_groups=[list(range(CORES))],
                ins=[xw0l[:].opt()],
                outs=[xw0f[:].opt()],
            )

            # phase D: build the A stripe in SBUF, one 128-gene window at a
            # time, as sums of one-hot outer products on the PE.  Also emits
            # the row-degree partials (free-axis rowsums).
            a_sb = []
            rd_sb = cpool.tile([128, ST], F32)
            for t in range(ST):
                pa = ppool.tile([128, OC], F32, tag="bld", bufs=2, name=f"pa{t}")
                for i in range(WCH):
                    c = t * WCH + i
                    loh = wpool.tile([128, 128], F16, tag="loh", bufs=3,
                                     name=f"loh{c}")
                    roh = wpool.tile([128, OC], F16, tag="roh", bufs=3,
                                     name=f"roh{c}")
                    nc.vector.tensor_scalar(
                        out=loh[:], in0=i128_sb[:],
                        scalar1=sloc_sb[:, c:c + 1], scalar2=None,
                        op0=mybir.AluOpType.is_equal,
                    )
                    nc.vector.tensor_scalar(
                        out=roh[:], in0=i512_sb[:],
                        scalar1=dloc_sb[:, c:c + 1], scalar2=None,
                        op0=mybir.AluOpType.is_equal,
                    )
                    nc.tensor.matmul(
                        pa[:], loh[:], roh[:],
                        start=(i == 0), stop=(i == WCH - 1),
                    )
                a_t = apool.tile([128, OC], F32, tag="A", name=f"a{t}")
                nc.scalar.copy(a_t[:], pa[:])
                a_sb.append(a_t)
                if debug_outputs:
                    nc.sync.dma_start(Adbg[t * 128:(t + 1) * 128, :], a_t[:])
                nc.vector.reduce_sum(
                    rd_sb[:, t:t + 1], a_t[:], axis=mybir.AxisListType.X
                )

            # col_deg = ones^T @ A  ([1, 512] psum accumulated over windows)
            pcd = ppool.tile([1, OC], F32, tag="cd", bufs=1)
            for t in range(ST):
                nc.tensor.matmul(
                    pcd[:], ones_sb[:], a_sb[t][:],
                    start=(t == 0), stop=(t == ST - 1),
                )
            cd_row = cpool.tile([1, OC], F32)
            nc.vector.tensor_copy(cd_row[:], pcd[:])
            # redistribute [1, 512] -> [128, 4]: column dt holds drugs
            # dt*128 + p on partition p (matches phase G's per-partition g)
            cd_sb = cpool.tile([128, 4], F32)
            for kq in range(4):
                nc.sync.dma_start(
                    cd_sb[:, kq:kq + 1], cd_row[0:1, kq * 128:(kq + 1) * 128]
                )

            # row_deg all-reduce and f = (deg>0)/sqrt(max(deg,1))
            nc.sync.dma_start(rdl[:], rd_sb[:])
            nc.gpsimd.collective_compute(
                "AllReduce",
                mybir.AluOpType.add,
                replica_groups=[list(range(CORES))],
                ins=[rdl[:].opt()],
                outs=[rds[:].opt()],
            )
            deg_sb = cpool.tile([128, ST], F32)
            nc.sync.dma_start(deg_sb[:], rds[:])
            t1 = cpool.tile([128, ST], F32)
            nc.vector.tensor_scalar(
                out=t1[:], in0=deg_sb[:], scalar1=1.0, scalar2=None,
                op0=mybir.AluOpType.max,
            )
            nc.scalar.sqrt(t1[:], t1[:])
            nc.vector.reciprocal(t1[:], t1[:])
            fmask = cpool.tile([128, ST], F32)
            nc.vector.tensor_scalar(
                out=fmask[:], in0=deg_sb[:], scalar1=0.5, scalar2=None,
                op0=mybir.AluOpType.is_gt,
            )
            f_sb = cpool.tile([128, ST], F32)
            nc.vector.tensor_tensor(
                out=f_sb[:], in0=t1[:], in1=fmask[:], op=mybir.AluOpType.mult
            )

            # g = (coldeg>0)/sqrt(max(coldeg,1))   [128, 4]
            g1 = cpool.tile([128, 4], F32)
            nc.vector.tensor_scalar(
                out=g1[:], in0=cd_sb[:], scalar1=1.0, scalar2=None,
                op0=mybir.AluOpType.max,
            )
            nc.scalar.sqrt(g1[:], g1[:])
            nc.vector.reciprocal(g1[:], g1[:])
            gmask = cpool.tile([128, 4], F32)
            nc.vector.tensor_scalar(
                out=gmask[:], in0=cd_sb[:], scalar1=0.5, scalar2=None,
                op0=mybir.AluOpType.is_gt,
            )
            g_sb = cpool.tile([128, 4], F32)
            nc.vector.tensor_tensor(
                out=g_sb[:], in0=g1[:], in1=gmask[:], op=mybir.AluOpType.mult
            )

            # phase F: out = (f*A)^T @ xw0f  accumulated over gene windows
            po = [ppool.tile([128, OC], F32, tag="acc", name=f"po{i}") for i in range(4)]
            for t in range(ST):
                nc.vector.tensor_scalar(
                    out=a_sb[t][:], in0=a_sb[t][:],
                    scalar1=f_sb[:, t:t + 1], scalar2=None,
                    op0=mybir.AluOpType.mult,
                )
                xf_t = wpool.tile([128, OC], F32, tag="xwf", bufs=3, name=f"xf{t}")
                nc.sync.dma_start(xf_t[:], xw0f[t * 128:(t + 1) * 128, :])
                if debug_outputs:
                    nc.sync.dma_start(xwdbg[t * 128:(t + 1) * 128, :], xf_t[:])
                for dt in range(4):
                    nc.tensor.matmul(
                        po[dt][:],
                        a_sb[t][:, dt * 128:(dt + 1) * 128],
                        xf_t[:],
                        start=(t == 0),
                        stop=(t == ST - 1),
                    )

            # phase G: scale by g, add bias, store
            for dt in range(4):
                og = wpool.tile([128, OC], F32, tag="og", name=f"og{dt}")
                nc.vector.tensor_scalar(
                    out=og[:], in0=po[dt][:],
                    scalar1=g_sb[:, dt:dt + 1], scalar2=None,
                    op0=mybir.AluOpType.mult,
                )
                nc.vector.tensor_tensor(
                    out=og[:], in0=og[:], in1=bias_sb[:], op=mybir.AluOpType.add
                )
                nc.sync.dma_start(out[dt * 128:(dt + 1) * 128, :], og[:])

    nc.finalize()
    return nc


def make_in_maps(x, weight, bias, edge_index):
    """Host-side sharding/layout only: no arithmetic on tensor values."""
    x = np.asarray(x, dtype=np.float32)
    weight = np.ascontiguousarray(np.asarray(weight, dtype=np.float32))
    bias = np.asarray(bias, dtype=np.float32)
    ei = np.asarray(edge_index)
    s_all = ei[0].astype(np.int64)
    d_all = ei[1].astype(np.int64)
    assert s_all.min() >= 0 and s_all.max() < ND, "src ids out of supported range"
    assert d_all.min() >= 0 and d_all.max() < ND, "dst ids out of supported range"

    brep = np.ascontiguousarray(np.tile(bias[None, :], (128, 1)).astype(np.float32))
    i128 = np.ascontiguousarray(
        np.tile(np.arange(128, dtype=np.float16)[None, :], (128, 1))
    )
    i512 = np.ascontiguousarray(
        np.tile(np.arange(OC, dtype=np.float16)[None, :], (128, 1))
    )

    core_of = d_all >> 9
    in_maps = []
    for c in range(CORES):
        m = core_of == c
        s = s_all[m]
        dl = d_all[m] - c * DWIN

        # window-major slot packing: gene window w = s >> 7 gets WCH chunks
        # of 128 slots; pads get -1 (all-zero one-hots)
        sl_lin = np.full(NSLOT, -1.0, dtype=np.float32)
        dl_lin = np.full(NSLOT, -1.0, dtype=np.float32)
        o = np.argsort(s, kind="stable")
        s_o = s[o]
        dl_o = dl[o]
        wnd = s_o >> 7
        cnt = np.bincount(wnd, minlength=ST)
        assert cnt.max() <= WCH * 128, f"window overflow: {cnt.max()}"
        pos = 0
        for t in range(ST):
            n = int(cnt[t])
            base = t * WCH * 128
            sl_lin[base:base + n] = (s_o[pos:pos + n] - t * 128).astype(np.float32)
            dl_lin[base:base + n] = dl_o[pos:pos + n].astype(np.float32)
            pos += n

        sloc_t = np.ascontiguousarray(sl_lin.reshape(NCH, 128).T)
        dloc_t = np.ascontiguousarray(dl_lin.reshape(NCH, 128).T)

        xsT = np.ascontiguousarray(x[c * DWIN:(c + 1) * DWIN, :].T)

        in_maps.append(
            {
                "xT": xsT,
                "w": weight,
                "brep": brep,
                "i128": i128,
                "i512": i512,
                "sloc": sloc_t,
                "dloc": dloc_t,
            }
        )
    return in_maps


_NC = None


def _get_nc():
    global _NC
    if _NC is None:
        _NC = build_nc()
    return _NC


def kernel(x, weight, bias, edge_index, **run_kwargs):
    from concourse.bass_utils import run_bass_kernel_spmd

    nc = _get_nc()
    in_maps = make_in_maps(x, weight, bias, edge_index)
    res = run_bass_kernel_spmd(nc, in_maps, core_ids=list(range(CORES)), **run_kwargs)
    outs = res.results if hasattr(res, "results") else res
    full = np.empty((ND, OC), dtype=np.float32)
    for c in range(CORES):
        n = min(DWIN, ND - c * DWIN)
        full[c * DWIN:c * DWIN + n] = outs[c]["out"][:n]
    if run_kwargs:
        return full, res
    return full
